# revision 90
# baseline (speedup 1.0000x reference)
"""Local (sliding-window) self-attention Bass kernel for 8 TRN2 NeuronCores.

Problem: B=4, T=4096, C=512, H=8 heads, head_dim=64, window=15.
Sharding: 8 cores = batch(4) x seq-halves(2). Each core processes 2048 query
tokens of one batch element; its x chunk carries a 7-token halo on each side
(zero-padded at sequence edges, matching the reference's jnp.pad semantics),
padded to 2080 rows for DMA alignment.

Per-core dataflow (bf16 matmuls, fp32 PSUM accumulation):
  x tile --mask*cast--> xb bf16 --one 3D XBAR transpose--> xT [128,4,2080]
  qT/kT feature-major GEMMs (bias on DVE), v token-major GEMM into
  overlapping 128-row tiles at 114 stride with a ones column per head
  (v_sb [128, 8, 65]) so AV yields the softmax denominator for free.
  Per 114-query block x head (software-pipelined, scores run 4 ahead
  of AV so the in-order TensorE queue never stalls on exp/band):
    scoresT [128k, 114q] = kT.T @ qT  -> exp on ACT -> band-mask on DVE
    -> AV (alb stationary) -> att_tok [114q, 65] token-major, col 64 =
    denominator -> reciprocal [114,1] -> ACT Copy scale=recip.
  One XBAR transpose per block -> aT3 feature-major -> proj GEMM +
  (bias, mask) DVE epilogue -> out DMA.
Emission is interleaved by x-tile readiness: v/q/k chunk GEMMs and then
attention+proj blocks are emitted as soon as their inputs exist, keeping
TensorE fed from ~15us instead of idling through the input phase.
"""

import math
from contextlib import ExitStack

import ml_dtypes
import numpy as np

import concourse.bacc as bacc
import concourse.bass as bass
import concourse.mybir as mybir
import concourse.tile as tile
from concourse import bass_utils

B, T, C, H, WIN = 4, 4096, 512, 8, 15
D = C // H            # 64
PAD = WIN // 2        # 7
NTOK = T // 2         # 2048 query tokens per core
NKV = 2080            # kv rows per core: 7 + 2048 + 7 = 2062, padded to 2080
QB = 114              # queries per attention block (keys fit 128 partitions)
NQB = 18              # 17 * 114 + 110 = 2048
QCH = [512, 512, 512, 288, 224]   # q-token chunks (finer tail -> earlier attn)
KCH = [512, 512, 512, 512, 32]    # kv token chunks
SCALE = math.log(WIN) / D
F32 = mybir.dt.float32
BF16 = mybir.dt.bfloat16


def _bandT() -> np.ndarray:
    """[128,114] band: bandT[k, q] = 1 iff q <= k <= q+14 (key-major)."""
    k = np.arange(128)[:, None]
    q = np.arange(QB)[None, :]
    return ((k >= q) & (k <= q + WIN - 1)).astype(ml_dtypes.bfloat16)


def build_program() -> bacc.Bacc:
    nc = bacc.Bacc("TRN2", target_bir_lowering=False, debug=False,
                   enable_asserts=False, num_devices=8)

    xd = nc.dram_tensor("x", [NKV, C], F32, kind="ExternalInput").ap()
    maskd = nc.dram_tensor("mask", [NKV], F32, kind="ExternalInput").ap()
    wqd = nc.dram_tensor("wq", [C, C], F32, kind="ExternalInput").ap()
    bqd = nc.dram_tensor("bq", [C], F32, kind="ExternalInput").ap()
    wkvd = nc.dram_tensor("wkv", [C, 2 * C], F32, kind="ExternalInput").ap()
    bkvd = nc.dram_tensor("bkv", [2 * C], F32, kind="ExternalInput").ap()
    wpd = nc.dram_tensor("wproj", [C, C], F32, kind="ExternalInput").ap()
    bpd = nc.dram_tensor("bproj", [C], F32, kind="ExternalInput").ap()
    bandd = nc.dram_tensor("bandt", [128, 4 * QB], BF16, kind="ExternalInput").ap()
    outd = nc.dram_tensor("out", [NTOK, C], F32, kind="ExternalOutput").ap()

    with tile.TileContext(nc) as tc, ExitStack() as ctx:
        sb = ctx.enter_context(tc.tile_pool(name="sb", bufs=1))
        sb_x = ctx.enter_context(tc.tile_pool(name="sb_x", bufs=5))
        sb_a = ctx.enter_context(tc.tile_pool(name="sb_a", bufs=12))
        sb_o = ctx.enter_context(tc.tile_pool(name="sb_o", bufs=4))
        pp_big = ctx.enter_context(tc.tile_pool(name="pp_big", bufs=2, space="PSUM"))
        pp_sc = ctx.enter_context(tc.tile_pool(name="pp_sc", bufs=4, space="PSUM"))
        pp_at = ctx.enter_context(tc.tile_pool(name="pp_at", bufs=2, space="PSUM"))

        # ---- persistent SBUF tensors ----
        xT = sb.tile([128, 4, NKV], BF16, tag="xT", name="xT")
        qT = [sb.tile([128, NTOK], BF16, tag=f"qT{i}", name=f"qT{i}") for i in range(4)]
        kT = [sb.tile([128, NKV], BF16, tag=f"kT{i}", name=f"kT{i}") for i in range(4)]
        v_sb = [sb.tile([128, 8, D + 1], BF16, tag=f"vsb{i}", name=f"vsb{i}")
                for i in range(NQB)]
        aT3 = sb.tile([128, 4, NQB * 128], BF16, tag="aT3", name="aT3")
        band4 = sb.tile([128, 4 * QB], BF16, tag="band4")
        wq = [sb.tile([128, C], BF16, tag=f"wq{i}", name=f"wq{i}") for i in range(4)]
        wk = [sb.tile([128, C], BF16, tag=f"wk{i}", name=f"wk{i}") for i in range(4)]
        wv = [sb.tile([128, C], BF16, tag=f"wv{i}", name=f"wv{i}") for i in range(4)]
        wp = [sb.tile([128, C], BF16, tag=f"wp{i}", name=f"wp{i}") for i in range(4)]
        # persistent f32 staging so all weight DMAs issue immediately
        wkf = [sb.tile([128, C], F32, tag=f"wkf{i}", name=f"wkf{i}") for i in range(4)]
        wvf = [sb.tile([128, C], F32, tag=f"wvf{i}", name=f"wvf{i}") for i in range(4)]
        wqf = [sb.tile([128, C], F32, tag=f"wqf{i}", name=f"wqf{i}") for i in range(4)]
        wpf = [sb.tile([128, C], F32, tag=f"wpf{i}", name=f"wpf{i}") for i in range(4)]
        bq_t = sb.tile([128, 4], F32, tag="bq")       # per-partition q bias
        bk_t = sb.tile([128, 4], F32, tag="bk")       # per-partition k bias
        bvB = sb.tile([128, C], F32, tag="bvB")       # v bias bcast over partitions
        bpB = sb.tile([128, C], F32, tag="bpB")       # proj bias bcast
        mq2 = sb.tile([128, NQB], F32, tag="mq2")     # query mask per block
        mqa = sb.tile([128, 17], F32, tag="mqa")      # mask per 128-row x tile
        att_toks = [sb.tile([128, C], BF16, tag=f"attok{i}", name=f"attok{i}")
                    for i in range(6)]                # per-block ring
        for i in range(6):
            nc.gpsimd.memset(att_toks[i][96:128, :], 0.0)

        # ---- const DMAs first (tiny; unblock the x pipeline), then weights ----
        nc.scalar.dma_start(mqa[:, 0:16],
                            maskd[0:2048].rearrange("(a b) -> b a", b=128))
        nc.scalar.dma_start(mqa[0:32, 16:17], maskd[2048:NKV][:, None])
        for ci in range(4):
            nc.scalar.dma_start(wvf[ci][:], wkvd[ci * 128:(ci + 1) * 128, C:2 * C])
        nc.scalar.dma_start(band4[:], bandd)
        nc.scalar.dma_start(bq_t[:], bqd.rearrange("(a b) -> b a", b=128))
        nc.scalar.dma_start(bk_t[:], bkvd[0:C].rearrange("(a b) -> b a", b=128))
        nc.scalar.dma_start(bvB[:], bkvd[C:2 * C][None, :].broadcast_to((128, C)))
        for ci in range(4):
            nc.scalar.dma_start(wkf[ci][:], wkvd[ci * 128:(ci + 1) * 128, 0:C])
        for ci in range(4):
            nc.scalar.dma_start(wqf[ci][:], wqd[ci * 128:(ci + 1) * 128, :])
        nc.scalar.dma_start(bpB[:], bpd[None, :].broadcast_to((128, C)))
        nc.scalar.dma_start(mq2[0:QB, 0:NQB - 1],
                            maskd[PAD:PAD + QB * (NQB - 1)].rearrange(
                                "(a b) -> b a", b=QB))
        nc.scalar.dma_start(mq2[0:NTOK - QB * (NQB - 1), NQB - 1:NQB],
                            maskd[PAD + QB * (NQB - 1):PAD + NTOK][:, None])
        for ci in range(4):
            nc.scalar.dma_start(wpf[ci][:], wpd[ci * 128:(ci + 1) * 128, :])

        # weight casts are deferred into the x loop (emit_wcast) so the
        # first xb casts aren't stuck behind them in the vector queue
        def emit_wcast(t: int):
            if t == 1:
                for ci in range(4):
                    nc.vector.tensor_copy(wv[ci][:], wvf[ci][:])
            elif t == 2:
                for ci in range(4):
                    nc.vector.tensor_copy(wk[ci][:], wkf[ci][:])
            elif t == 3:
                for ci in range(4):
                    nc.vector.tensor_copy(wq[ci][:], wqf[ci][:])
            elif t == 5:
                for ci in range(4):
                    nc.vector.tensor_copy(wp[ci][:], wpf[ci][:])

        # ---- emission helpers ----
        def emit_v(j: int):
            r0 = j * QB
            nc.gpsimd.memset(v_sb[j][:, :, D:D + 1], 1.0)
            ps = pp_big.tile([128, 512], F32, tag="big", name="psv")
            for ci in range(4):
                nc.tensor.matmul(
                    ps[:], xT[:, ci, r0:r0 + 128],
                    wv[ci][:], start=(ci == 0), stop=(ci == 3))
            nc.vector.scalar_tensor_tensor(
                v_sb[j][:, :, 0:D],
                ps.rearrange("p (h c) -> p h c", c=D), 1.0,
                bvB.rearrange("p (h c) -> p h c", c=D),
                op0=mybir.AluOpType.mult, op1=mybir.AluOpType.add)

        def emit_q(ch: int):
            t0 = sum(QCH[:ch])
            w = QCH[ch]
            for co in range(4):
                ps = pp_big.tile([128, 512], F32, tag="big", name="psq")
                for ci in range(4):
                    nc.tensor.matmul(
                        ps[:, 0:w], wq[ci][:, co * 128:(co + 1) * 128],
                        xT[:, ci, PAD + t0:PAD + t0 + w],
                        start=(ci == 0), stop=(ci == 3))
                nc.vector.tensor_scalar_add(qT[co][:, t0:t0 + w], ps[:, 0:w],
                                            bq_t[:, co:co + 1])

        def emit_k(ch: int):
            t0 = sum(KCH[:ch])
            w = KCH[ch]
            for co in range(4):
                ps = pp_big.tile([128, 512], F32, tag="big", name="psk")
                for ci in range(4):
                    nc.tensor.matmul(
                        ps[:, 0:w], wk[ci][:, co * 128:(co + 1) * 128],
                        xT[:, ci, t0:t0 + w],
                        start=(ci == 0), stop=(ci == 3))
                nc.vector.tensor_scalar_add(kT[co][:, t0:t0 + w], ps[:, 0:w],
                                            bk_t[:, co:co + 1])

        # ---- attention group pipeline (cross-block sc lookahead) ----
        LOOKAHEAD = 3
        groups: list[tuple[int, int]] = []   # (block j, group g)
        g_albs: dict[tuple[int, int], bass.AP] = {}
        sc_idx = 0
        av_idx = 0

        def emit_sc(j: int, g: int):
            qb = min(QB, NTOK - j * QB)
            q0 = j * QB
            al4 = sb_a.tile([128, 4 * QB], BF16, tag="al4", name="al4")
            for i in range(4):
                h = 4 * g + i
                hp, hh = divmod(h, 2)
                sc = pp_sc.tile([128, QB], F32, tag="sc", name="sc")
                nc.tensor.matmul(
                    sc[:, 0:qb],
                    kT[hp][hh * 64:(hh + 1) * 64, q0:q0 + 128],
                    qT[hp][hh * 64:(hh + 1) * 64, q0:q0 + qb],
                    start=True, stop=True)
                nc.scalar.activation(al4[:, i * qb:(i + 1) * qb],
                                     sc[:, 0:qb],
                                     mybir.ActivationFunctionType.Exp,
                                     scale=SCALE)
            alb4 = sb_a.tile([128, 4 * QB], BF16, tag="alb4", name="alb4")
            nc.vector.scalar_tensor_tensor(
                alb4[:, 0:4 * qb].rearrange("p (g q) -> p g q", q=qb),
                al4[:, 0:4 * qb].rearrange("p (g q) -> p g q", q=qb), 1.0,
                band4.rearrange("p (g q) -> p g q", q=QB)[:, :, 0:qb],
                op0=mybir.AluOpType.mult, op1=mybir.AluOpType.mult)
            g_albs[(j, g)] = alb4

        def emit_av(j: int, g: int):
            qb = min(QB, NTOK - j * QB)
            alb4 = g_albs.pop((j, g))
            att_tok = att_toks[j % 6]
            at4 = pp_at.tile([QB, 4 * (D + 1)], F32, tag="at4", name="at4")
            for i in range(4):
                h = 4 * g + i
                nc.tensor.matmul(
                    at4[0:qb, i * (D + 1):(i + 1) * (D + 1)],
                    alb4[:, i * qb:(i + 1) * qb], v_sb[j][:, h, :],
                    start=True, stop=True)
            rdq4 = sb_a.tile([QB, 4], F32, tag="rdq4", name="rdq4")
            nc.vector.reciprocal(rdq4[0:qb, :],
                                 at4[0:qb, D:4 * (D + 1):D + 1])
            nc.vector.scalar_tensor_tensor(
                att_tok[0:qb, g * 256:(g + 1) * 256].rearrange(
                    "p (h c) -> p h c", c=D),
                at4[0:qb, :].rearrange("p (h c) -> p h c", c=D + 1)[:, :, 0:D],
                1.0,
                rdq4[0:qb, :, None].broadcast_to((qb, 4, D)),
                op0=mybir.AluOpType.mult, op1=mybir.AluOpType.mult)
            if j >= NQB - 3:
                # tail blocks: half-transpose per group so proj's first ci
                # matmuls aren't serialized behind the full transpose
                eng = nc.scalar if (j + g) % 2 == 0 else nc.sync
                eng.dma_start_transpose(
                    aT3[:, 2 * g:2 * g + 2, j * 128:(j + 1) * 128],
                    att_toks[j % 6][:, g * 256:(g + 1) * 256])
            elif g == 1:
                eng = nc.scalar if j % 2 == 0 else nc.sync
                eng.dma_start_transpose(aT3[:, :, j * 128:(j + 1) * 128],
                                        att_toks[j % 6][:, :])

        def pump_attn(force: bool = False):
            nonlocal sc_idx, av_idx
            while sc_idx < len(groups) and sc_idx <= av_idx + LOOKAHEAD:
                emit_sc(*groups[sc_idx])
                sc_idx += 1
            while av_idx < len(groups) and (
                    force or av_idx < sc_idx - LOOKAHEAD):
                jj, gg = groups[av_idx]
                emit_av(jj, gg)
                av_idx += 1
                if gg == 1:
                    proj_pend.append(jj)
                    if len(proj_pend) > 1:
                        emit_proj(proj_pend.pop(0))
                while sc_idx < len(groups) and sc_idx <= av_idx + LOOKAHEAD:
                    emit_sc(*groups[sc_idx])
                    sc_idx += 1

        def emit_attn(j: int):
            groups.append((j, 0))
            groups.append((j, 1))
            pump_attn()

        def emit_proj(j: int):
            qb = min(QB, NTOK - j * QB)
            q0 = j * QB
            ps = pp_big.tile([128, 512], F32, tag="big", name="psp")
            for ci in range(4):
                nc.tensor.matmul(
                    ps[0:qb, :], aT3[:, ci, j * 128:j * 128 + qb],
                    wp[ci][:], start=(ci == 0), stop=(ci == 3))
            ot = sb_o.tile([128, C], F32, tag="ot", name="ot")
            nc.vector.scalar_tensor_tensor(
                ot[0:qb, :], ps[0:qb, :], 1.0, bpB[0:qb, :],
                op0=mybir.AluOpType.mult, op1=mybir.AluOpType.add)
            nc.vector.tensor_scalar_mul(ot[0:qb, :], ot[0:qb, :],
                                        mq2[0:qb, j:j + 1])
            nc.scalar.dma_start(outd[q0:q0 + qb, :], ot[0:qb, :])

        # readiness (last x tile needed) for each unit of work
        tv = {j: (j * QB + 127) // 128 for j in range(NQB)}
        tq = {ch: (PAD + sum(QCH[:ch + 1]) - 1) // 128 for ch in range(len(QCH))}
        tk = {ch: (sum(KCH[:ch + 1]) - 1) // 128 for ch in range(len(KCH))}

        def q_ch_of(c):  # q chunk covering column c
            for ch in range(len(QCH)):
                if c < sum(QCH[:ch + 1]):
                    return ch
            return len(QCH) - 1

        ta = {}
        for j in range(NQB):
            qb = min(QB, NTOK - j * QB)
            ch_k = (j * QB + 127) // 512
            ta[j] = max(tv[j], tq[q_ch_of(j * QB + qb - 1)], tk[ch_k])

        # ---- x pipeline interleaved with compute by tile readiness ----
        attn_ready: list[int] = []
        proj_pend: list[int] = []

        def drain_attn(budget: int):
            while attn_ready and budget > 0:
                emit_attn(attn_ready.pop(0))
                budget -= 1

        # skewed stages: DMA tile t, cast tile t-1, transpose tile t-2 —
        # casts stay ahead of attention DVE work in the vector queue
        xfs: dict[int, bass.AP] = {}
        xbs: dict[int, bass.AP] = {}
        for t in range(19):
            if t < 17:
                r0, r1 = t * 128, min((t + 1) * 128, NKV)
                xf = sb_x.tile([128, C], F32, tag="xf", name="xf")
                nc.gpsimd.dma_start(xf[:r1 - r0, :], xd[r0:r1, :])
                xfs[t] = xf
            if 1 <= t <= 17:
                tc_ = t - 1
                r0, r1 = tc_ * 128, min((tc_ + 1) * 128, NKV)
                xb = sb_x.tile([128, C], BF16, tag="xb", name="xb")
                nc.vector.tensor_scalar_mul(xb[:r1 - r0, :],
                                            xfs.pop(tc_)[:r1 - r0, :],
                                            mqa[:r1 - r0, tc_:tc_ + 1])
                xbs[tc_] = xb
            emit_wcast(t)
            if t < 2:
                continue
            tt = t - 2
            r0, r1 = tt * 128, min((tt + 1) * 128, NKV)
            nc.sync.dma_start_transpose(xT[:, :, r0:r1],
                                        xbs.pop(tt)[:r1 - r0, :])

            for j in range(NQB):
                if tv[j] == tt:
                    emit_v(j)
            for ch in range(len(QCH)):
                if tq[ch] == tt:
                    emit_q(ch)
            for ch in range(len(KCH)):
                if tk[ch] == tt:
                    emit_k(ch)
            for j in range(NQB):
                if ta[j] == tt:
                    attn_ready.append(j)
            drain_attn((2 if tt < 12 else 3) if tt < 16 else len(attn_ready))
        drain_attn(len(attn_ready))
        pump_attn(force=True)
        while proj_pend:
            emit_proj(proj_pend.pop(0))

    nc.compile()
    return nc


_CACHE: dict = {}


def _get_program() -> bacc.Bacc:
    if "nc" not in _CACHE:
        _CACHE["nc"] = build_program()
    return _CACHE["nc"]


def kernel(x, mask, Wq, bq, Wkv, bkv, Wproj, bproj) -> np.ndarray:
    x = np.asarray(x, np.float32)
    mask = np.asarray(mask, np.float32)
    bandt = np.ascontiguousarray(np.tile(_bandT(), (1, 4)))
    nc = _get_program()

    in_maps = []
    for core in range(8):
        b, h = divmod(core, 2)
        s = h * NTOK
        xc = np.zeros((NKV, C), np.float32)
        mc = np.zeros((NKV,), np.float32)
        lo, hi = max(0, s - PAD), min(T, s + NTOK + PAD)
        xc[lo - (s - PAD):lo - (s - PAD) + hi - lo] = x[b, lo:hi]
        mc[lo - (s - PAD):lo - (s - PAD) + hi - lo] = mask[b, lo:hi]
        in_maps.append({
            "x": xc, "mask": mc,
            "wq": np.asarray(Wq, np.float32), "bq": np.asarray(bq, np.float32),
            "wkv": np.asarray(Wkv, np.float32), "bkv": np.asarray(bkv, np.float32),
            "wproj": np.asarray(Wproj, np.float32),
            "bproj": np.asarray(bproj, np.float32),
            "bandt": bandt,
        })

    res = bass_utils.run_bass_kernel_spmd(nc, in_maps, core_ids=list(range(8)))
    out = np.empty((B, T, C), np.float32)
    for core in range(8):
        b, h = divmod(core, 2)
        out[b, h * NTOK:(h + 1) * NTOK] = res.results[core]["out"]
    return out
